# revision 1
# baseline (speedup 1.0000x reference)
"""Brute-force KNN density estimator on 8 Trainium2 NeuronCores.

reference math:
    dist[i, j] = ||x_i - x_j||_2 over features [8192, 1024]
    kth[i] = 6th smallest of dist[i, :]  (self-distance included)
    out[i] = 1 / (kth[i] + 1e-8)

Strategy (data-parallel over query rows, 1024 rows per core):
    - Rank rows of the distance matrix by T[i,j] = 2*G[i,j] - (sq[j] - mean(sq))
      (per-row-constant sq[i] and the monotone sqrt don't change ranking).
    - TensorE: G via fp8 e4m3 DoubleRow matmuls (2x MAC throughput, fp32
      PSUM accumulation); the norm subtraction is folded into the same
      accumulation group as one bf16 matmul with lhsT = -1/128 constant
      and rhs = centered norms replicated across partitions.
    - VectorE: single MAX8 per [128, 512] PSUM tile -> per-tile top-8
      candidates; final MAX8 over candidates gives the exact 6th largest
      T, recovered to a distance with exact fp32 norms on the host side:
      kth_d2 = (sq[i] + mean(sq)) - T6.
"""

import os

import numpy as np
import ml_dtypes

N = 8192          # points
D = 1024          # feature dim
NCORES = 8
ROWS = N // NCORES   # rows (queries) per core
RT = ROWS // 128     # row tiles per core
CTILE = 512          # matmul moving free dim
CT = N // CTILE      # column tiles
KC = D // 128        # 128-row contraction chunks
K_ORD = 5            # 0-based rank -> 6th smallest
EPS = 1e-8
WARMUP_MM = 18       # dummy matmuls to trigger the PE HAM warm clock early

TRACE = bool(int(os.environ.get("KNN_TRACE", "0")))
LAST_EXEC_NS = None


def _build_nc():
    import concourse.mybir as mybir
    from concourse import bacc
    from concourse.tile import TileContext

    dt = mybir.dt
    nc = bacc.Bacc(None, target_bir_lowering=False, enable_partition_id=False)

    # per-tile layout [CT][128 part][KC*CTILE contiguous] -> one DMA per tile
    ft_d = nc.dram_tensor("ft", [CT, 128, KC * CTILE], dt.float8e4, kind="ExternalInput")
    qt_d = nc.dram_tensor("qt", [128, KC * ROWS], dt.float8e4, kind="ExternalInput")
    sqc_d = nc.dram_tensor("sqc", [128, N], dt.bfloat16, kind="ExternalInput")
    sqi_d = nc.dram_tensor("sqi", [128, RT], dt.float32, kind="ExternalInput")
    out_d = nc.dram_tensor("out", [128, RT], dt.float32, kind="ExternalOutput")

    DR = mybir.MatmulPerfMode.DoubleRow

    with TileContext(nc) as tc:
        with (
            tc.tile_pool(name="persist", bufs=1) as persist,
            tc.tile_pool(name="ftp", bufs=3) as ftp,
            tc.tile_pool(name="small", bufs=2) as small,
            tc.tile_pool(name="psum", bufs=8, space="PSUM") as psum,
        ):
            qt_s = persist.tile([128, KC, ROWS], dt.float8e4)
            sqc_s = persist.tile([128, N], dt.bfloat16)
            sqi_s = persist.tile([128, RT], dt.float32)
            cand = persist.tile([128, RT * CT * 8], dt.float32)
            top8s = persist.tile([128, RT, 8], dt.float32)
            neg_s = persist.tile([128, 128], dt.bfloat16)
            warm_s = persist.tile([128, CTILE], dt.bfloat16)

            # PE warm-up: keep the PE busy during the initial DMA window so
            # the HAM clock gate reaches 2.4 GHz before the real matmuls
            nc.vector.memset(neg_s, -1.0 / 128.0)
            nc.vector.memset(warm_s, 0.0)
            wps = psum.tile([128, CTILE], dt.float32, tag="ps")
            for i in range(WARMUP_MM):
                nc.tensor.matmul(wps, lhsT=neg_s, rhs=warm_s,
                                 start=(i == 0), stop=(i == WARMUP_MM - 1))

            ft_tiles = []
            ft_t0 = ftp.tile([128, KC, CTILE], dt.float8e4, tag="ft")
            nc.sync.dma_start(ft_t0, ft_d[0].rearrange("p (k j) -> p k j", k=KC))
            ft_tiles.append(ft_t0)
            nc.sync.dma_start(qt_s, qt_d[:, :].rearrange("p (k i) -> p k i", k=KC))
            for t in range(1, 3):  # prefetch the next two column tiles
                ft_t = ftp.tile([128, KC, CTILE], dt.float8e4, tag="ft")
                nc.sync.dma_start(ft_t, ft_d[t].rearrange("p (k j) -> p k j", k=KC))
                ft_tiles.append(ft_t)
            nc.sync.dma_start(sqi_s, sqi_d[:, :])
            for t in range(CT):
                nc.sync.dma_start(
                    sqc_s[:, t * CTILE:(t + 1) * CTILE],
                    sqc_d[:, t * CTILE:(t + 1) * CTILE],
                )

            for t in range(CT):
                if t < 3:
                    ft_t = ft_tiles[t]
                else:
                    ft_t = ftp.tile([128, KC, CTILE], dt.float8e4, tag="ft")
                    nc.sync.dma_start(ft_t, ft_d[t].rearrange("p (k j) -> p k j", k=KC))
                sqc_t = sqc_s[:, t * CTILE:(t + 1) * CTILE]
                for r in range(RT):
                    ps = psum.tile([128, CTILE], dt.float32, tag="ps")
                    for k in range(0, KC, 2):
                        nc.tensor.matmul(
                            ps,
                            lhsT=qt_s[:, k:k + 2, r * 128:(r + 1) * 128],
                            rhs=ft_t[:, k:k + 2, :],
                            start=(k == 0),
                            stop=False,
                            perf_mode=DR,
                        )
                    # T = 2G - sqc: rhs replicated across K partitions,
                    # scaled by lhsT = -1/128
                    nc.tensor.matmul(ps, lhsT=neg_s, rhs=sqc_t,
                                     start=False, stop=True)
                    nc.vector.max(
                        out=cand[:, (r * CT + t) * 8:(r * CT + t + 1) * 8],
                        in_=ps,
                    )

            # batched finals: one [128, RT]-wide chain instead of RT chains
            for r in range(RT):
                nc.vector.max(out=top8s[:, r, :],
                              in_=cand[:, r * CT * 8:(r + 1) * CT * 8])
            kd = small.tile([128, RT], dt.float32, tag="kd")
            # T6 column per row-tile: stride-8 slice of top8s
            nc.vector.tensor_sub(kd, sqi_s, top8s[:, :, K_ORD])
            nc.vector.tensor_scalar_max(kd, kd, 0.0)
            ks = small.tile([128, RT], dt.float32, tag="ks")
            nc.scalar.activation(ks, kd, mybir.ActivationFunctionType.Sqrt)
            nc.vector.tensor_scalar_add(ks, ks, EPS)
            dens = small.tile([128, RT], dt.float32, tag="dens")
            nc.vector.reciprocal(dens, ks)
            nc.sync.dma_start(out_d[:, :], dens)

    # run Bacc's passes (register allocation, event-semaphore wait splitting)
    # before handing off to the PJRT path, which binds without finalizing
    nc.finalize()
    return nc


def kernel(features):
    global LAST_EXEC_NS
    from concourse.bass_utils import run_bass_kernel_spmd

    f32 = np.ascontiguousarray(np.asarray(features, dtype=np.float32))
    assert f32.shape == (N, D)

    sq = np.einsum("nd,nd->n", f32, f32, dtype=np.float32)   # exact fp32 norms
    sbar = float(sq.mean())
    ftq = f32.T.astype(ml_dtypes.float8_e4m3fn)               # [D, N] fp8
    # moving operand pre-scaled by 2 (exact in fp8) so PSUM accumulates 2*G
    ft2 = (ftq.astype(np.float32) * 2.0).astype(ml_dtypes.float8_e4m3fn)
    # [D, N] -> [CT, 128, KC*CTILE]: per column tile, partition p holds all
    # KC chunks contiguously -> a single fully-contiguous DMA per tile
    ft_tiles = np.ascontiguousarray(
        ft2.reshape(KC, 128, CT, CTILE).transpose(2, 1, 0, 3).reshape(CT, 128, KC * CTILE)
    )
    sqc_rep = np.ascontiguousarray(
        np.broadcast_to((sq - sbar).astype(ml_dtypes.bfloat16), (128, N))
    )

    in_maps = []
    for c in range(NCORES):
        lo = c * ROWS
        qt = np.ascontiguousarray(
            ftq[:, lo:lo + ROWS].reshape(KC, 128, ROWS).transpose(1, 0, 2).reshape(128, KC * ROWS)
        )
        sqi = np.ascontiguousarray(
            (sq[lo:lo + ROWS] + sbar).reshape(RT, 128).T.astype(np.float32)
        )
        in_maps.append({"ft": ft_tiles, "qt": qt, "sqc": sqc_rep, "sqi": sqi})

    nc = _build_nc()
    res = run_bass_kernel_spmd(nc, in_maps, core_ids=list(range(NCORES)), trace=TRACE)
    LAST_EXEC_NS = res.exec_time_ns

    # out[p, r] = density of global row  c*1024 + r*128 + p
    out = np.concatenate([r["out"].T.reshape(-1) for r in res.results])
    return out.astype(np.float32)[:, None]



# revision 2
# speedup vs baseline: 1.0830x; 1.0830x over previous
"""Symmetric brute-force KNN density estimator on 8 Trainium2 NeuronCores.

reference math:
    dist[i, j] = ||x_i - x_j||_2 over features [8192, 1024]
    kth[i] = 6th smallest of dist[i, :]  (self-distance included)
    out[i] = 1 / (kth[i] + 1e-8)

v3 strategy — full symmetry (circulant blocks + quadrant splits + host merge):
    Rank rows by T[i,j] = 2G[i,j] - (sq[j] - sbar); d2 = (sq[i]+sbar) - T.
    8 row-blocks of 1024. Core c computes, via fp8 DoubleRow matmuls:
      * blocks c+1..c+3 fully (24 [128,1024] PSUM groups), each mirrored;
      * the c+4 block's diagonal quadrants (8 [128,512] half-groups, all
        mirrored) — the paired core covers the anti-diagonal quadrants (its
        ft buffer has the halves swapped on the host);
      * the diagonal block's UL/LR quadrants fully plus UR mirrored
        (12 half-groups).
    Mirroring: the scalar engine copies raw-2G PSUM to SBUF bf16 adding the
    per-partition bias -(sq_i - sbar) (exactly the transposed tile's column
    bias), then dma_start_transpose scatters [128,128] pieces into mirror
    storage with a contiguous destination per source tile (a strided dest
    produces wrong output on HW). DVE max8 scans computed PSUM groups
    (FD=1024) and mirror banks (strided SBUF bf16 reads). The host merges
    80 top-8 candidate sets per core (10 sets per row, asserted) and does
    the sqrt/reciprocal recovery with exact fp32 norms.
"""

import os

import numpy as np
import ml_dtypes

N = 8192
D = 1024
NCORES = 8
NB = 8            # row/col blocks of 1024
BLK = N // NB     # 1024
KC = D // 128     # 8 contraction chunks -> 4 DoubleRow pairs
KP = KC // 2      # 4
RT = BLK // 128   # 8 row chunks per block
K_ORD = 5         # 6th largest/smallest
EPS = 1e-8
WARMUP_MM = 18
NSLOT = 5         # ft column-block slots: c+1, c+2, c+3, c+4, c(diag)
NMIR = 3          # full-block mirrored slots

# canonical device-order work lists (identical for every core; SPMD)
#   ('full', bi, r)          : [128,1024] group, slot bi in 0..2, mirrored
#   ('k4',   r)              : [128,512] group, slot 3, col off (r//4)*512, mirrored
#   ('dq',   r, off, mir)    : diag slot 4 half-group
GROUPS = (
    [("k4", r) for r in range(RT)]
    + [("dfull", r) for r in range(4)]
    + [("full", bi, r) for bi in range(NMIR) for r in range(RT)]
    + [("dq", r, 512, False) for r in range(4, 8)]
)
# mirror banks (scanned after their source transposes land). Full-block
# banks are split into r 0..3 / 4..7 halves so the first half drains while
# the block's later groups still run.
MBANKS = (
    [("mk", h, q) for h in range(2) for q in range(4)]
    + [("md", q) for q in range(4)]
    + [("mfa", bi, q) for bi in range(NMIR) for q in range(RT)]
    + [("mfb", bi, q) for bi in range(NMIR) for q in range(RT)]
)
N_COMP = len(GROUPS)            # 44
NSETS = N_COMP + len(MBANKS)    # 44 + 36 = 80

TRACE = bool(int(os.environ.get("KNN_TRACE", "0")))
LAST_EXEC_NS = None


def _build_nc():
    import concourse.mybir as mybir
    from concourse import bacc
    from concourse.tile import TileContext

    dt = mybir.dt
    nc = bacc.Bacc(None, target_bir_lowering=False, enable_partition_id=False)

    qt_d = nc.dram_tensor("qt", [128, KP * 2 * BLK], dt.float8e4, kind="ExternalInput")
    ft_d = nc.dram_tensor("ft", [NSLOT, 128, KP * 2 * BLK], dt.float8e4, kind="ExternalInput")
    sqc_d = nc.dram_tensor("sqc", [1, NSLOT * BLK], dt.bfloat16, kind="ExternalInput")
    sqa_d = nc.dram_tensor("sqa", [128, RT], dt.float32, kind="ExternalInput")
    cand_d = nc.dram_tensor("cand", [128, NSETS * 8], dt.float32, kind="ExternalOutput")

    DR = mybir.MatmulPerfMode.DoubleRow

    with TileContext(nc) as tc:
        with (
            tc.tile_pool(name="persist", bufs=1) as persist,
            tc.tile_pool(name="cp", bufs=3) as cpp,
            tc.tile_pool(name="cph", bufs=4) as cphp,
            tc.tile_pool(name="psum", bufs=4, space="PSUM") as psum,
        ):
            qt_s = persist.tile([128, KP, 2, BLK], dt.float8e4)
            ft_s = persist.tile([128, NSLOT, KP, 2, BLK], dt.float8e4)
            sqc_s = persist.tile([1, NSLOT * BLK], dt.bfloat16)
            sqa_s = persist.tile([128, RT], dt.float32)
            ones_s = persist.tile([1, 128], dt.bfloat16)
            warm_s = persist.tile([128, 512], dt.bfloat16)
            # contiguous-per-source-tile mirror storage (see module docstring)
            mir_f = persist.tile([128, NMIR, RT, RT, 128], dt.bfloat16)
            mir_k = persist.tile([128, RT, 4, 128], dt.bfloat16)
            mir_d = persist.tile([128, 4, 4, 128], dt.bfloat16)
            cand = persist.tile([128, NSETS * 8], dt.float32)

            # PE warm-up during the initial DMA window (HAM clock gate)
            nc.vector.memset(ones_s, 1.0)
            nc.vector.memset(warm_s, 0.0)
            wtile = psum.tile([128, 1024], dt.float32, tag="ps")
            for i in range(WARMUP_MM):
                nc.tensor.matmul(wtile[:, 0:512], lhsT=warm_s[:, 0:128], rhs=warm_s,
                                 start=(i == 0), stop=(i == WARMUP_MM - 1))

            qt_r = qt_d[:, :].rearrange("p (k t j) -> p k t j", k=KP, t=2)
            ft_r = [ft_d[b].rearrange("p (k t j) -> p k t j", k=KP, t=2)
                    for b in range(NSLOT)]
            nc.sync.dma_start(qt_s[:, 0:2], qt_r[:, 0:2])
            nc.sync.dma_start(ft_s[:, 3, 0:2], ft_r[3][:, 0:2])
            nc.sync.dma_start(qt_s[:, 2:4], qt_r[:, 2:4])
            nc.sync.dma_start(ft_s[:, 3, 2:4], ft_r[3][:, 2:4])
            nc.sync.dma_start(sqc_s, sqc_d[:, :])
            nc.sync.dma_start(sqa_s, sqa_d[:, :])
            nc.sync.dma_start(ft_s[:, 4], ft_r[4])
            nc.sync.dma_start(ft_s[:, 0], ft_r[0])
            nc.gpsimd.dma_start(ft_s[:, 1], ft_r[1])
            nc.gpsimd.dma_start(ft_s[:, 2], ft_r[2])

            # device emission must match the canonical order: computed sets
            # first (GROUPS order), then mirror banks (MBANKS order). Mirror
            # scans are interleaved for pipelining but their cand slots are
            # pre-assigned from the MBANKS order.
            comp_slots = {}
            for i, g in enumerate(GROUPS):
                comp_slots[g] = i
            bank_slots = {}
            for i, mb in enumerate(MBANKS):
                bank_slots[mb] = N_COMP + i

            def scan_bank_at(mb):
                s = bank_slots[mb]
                out = cand[:, s * 8:(s + 1) * 8]
                kind = mb[0]
                if kind == "mfa":
                    _, bi, q = mb
                    nc.vector.max(out=out, in_=mir_f[:, bi, 0:4, q, :])
                elif kind == "mfb":
                    _, bi, q = mb
                    nc.vector.max(out=out, in_=mir_f[:, bi, 4:8, q, :])
                elif kind == "mk":
                    _, h, q = mb
                    nc.vector.max(out=out, in_=mir_k[:, 4 * h:4 * h + 4, q, :])
                else:
                    _, q = mb
                    nc.vector.max(out=out, in_=mir_d[:, 0:4, q, :])

            tcount = [0]

            def do_group(g):
                kind = g[0]
                if kind == "full":
                    _, bi, r = g
                    slot, off, w, mir = bi, 0, 1024, True
                elif kind == "k4":
                    _, r = g
                    slot, off, w, mir = 3, (r // 4) * 512, 512, True
                elif kind == "dfull":
                    _, r = g
                    slot, off, w, mir = 4, 0, 1024, True
                else:
                    _, r, off, mir = g
                    slot, w = 4, 512
                ps = psum.tile([128, 1024], dt.float32, tag="ps")
                nh = w // 512
                for kp in range(KP):
                    for half in range(nh):
                        nc.tensor.matmul(
                            ps[:, half * 512:(half + 1) * 512],
                            lhsT=qt_s[:, kp, :, r * 128:(r + 1) * 128],
                            rhs=ft_s[:, slot, kp, :, off + half * 512: off + (half + 1) * 512],
                            start=(kp == 0), stop=False, perf_mode=DR)
                if mir:
                    if kind == "full":
                        cp = cpp.tile([128, 1024], dt.bfloat16, tag="cp")
                        nc.scalar.add(cp, ps, sqa_s[:, r:r + 1])
                        dst = mir_f[:, slot, r, :, :]
                    elif kind == "dfull":
                        # only the UR half (cols 512:1024) is mirrored -> LL
                        cp = cphp.tile([128, 512], dt.bfloat16, tag="cph")
                        nc.scalar.add(cp, ps[:, 512:1024], sqa_s[:, r:r + 1])
                        dst = mir_d[:, r, :, :]
                    else:
                        cp = cphp.tile([128, 512], dt.bfloat16, tag="cph")
                        nc.scalar.add(cp, ps[:, 0:512], sqa_s[:, r:r + 1])
                        dst = mir_k[:, r, :, :] if kind == "k4" else mir_d[:, r, :, :]
                    if kind == "full":
                        eng = nc.sync if (tcount[0] % 2 == 0) else nc.scalar
                        tcount[0] += 1
                    else:
                        eng = nc.sync
                    eng.dma_start_transpose(dst, cp)
                return (g, slot, off, w, ps)

            def finish_group(st):
                g, slot, off, w, ps = st
                nh = w // 512
                for half in range(nh):
                    c0 = slot * BLK + off + half * 512
                    nc.tensor.matmul(
                        ps[:, half * 512:(half + 1) * 512],
                        lhsT=ones_s, rhs=sqc_s[:, c0:c0 + 512],
                        start=False, stop=(half == nh - 1))
                s = comp_slots[g]
                nc.vector.max(out=cand[:, s * 8:(s + 1) * 8], in_=ps[:, 0:w])

            # schedule: emit groups; after each group, drain one pending bank
            ready_after = {}
            for bi in range(NMIR):
                ready_after[("full", bi, 3)] = [("mfa", bi, q) for q in range(RT)]
                ready_after[("full", bi, 7)] = [("mfb", bi, q) for q in range(RT)]
            ready_after[("k4", 3)] = [("mk", 0, q) for q in range(4)]
            ready_after[("k4", 7)] = [("mk", 1, q) for q in range(4)]
            ready_after[("dfull", 3)] = [("md", q) for q in range(4)]

            queue = []
            delay = []         # one-group delay before banks become poppable
            for g in GROUPS:
                st = do_group(g)
                finish_group(st)
                queue.extend(delay)
                delay = ready_after.get(g, [])
                npop = 2 if len(queue) >= 4 else 1
                for _ in range(min(npop, len(queue))):
                    scan_bank_at(queue.pop(0))
            queue.extend(delay)
            for mb in queue:
                scan_bank_at(mb)

            nc.sync.dma_start(cand_d[:, :], cand)

    nc.finalize()
    return nc


def kernel(features):
    global LAST_EXEC_NS
    from concourse.bass_utils import run_bass_kernel_spmd

    f32 = np.ascontiguousarray(np.asarray(features, dtype=np.float32))
    assert f32.shape == (N, D)

    sq = np.einsum("nd,nd->n", f32, f32, dtype=np.float32)
    sbar = float(sq.mean())

    ftq = f32.T.astype(ml_dtypes.float8_e4m3fn)                   # [D, N] fp8
    ft2 = (ftq.astype(np.float32) * 2.0).astype(ml_dtypes.float8_e4m3fn)
    sqd = (-(sq - sbar)).astype(np.float32)

    def chunk_cols(src, cols):
        blk = src[:, cols]                                        # [D, BLK]
        return blk.reshape(KP, 2, 128, BLK).transpose(2, 0, 1, 3).reshape(128, KP * 2 * BLK)

    in_maps = []
    col_tables = []
    for c in range(NCORES):
        blocks = [(c + o) % NB for o in [1, 2, 3, 4, 0]]
        # slot 3 (k4): swap column halves for c >= 4 so the shared device
        # program computes complementary quadrants on the two paired cores
        slot_cols = []
        for si, b in enumerate(blocks):
            cols = np.arange(b * BLK, (b + 1) * BLK)
            if si == 3 and c >= 4:
                cols = np.concatenate([cols[512:], cols[:512]])
            slot_cols.append(cols)
        col_tables.append(slot_cols)
        qt = np.ascontiguousarray(chunk_cols(ftq, np.arange(c * BLK, (c + 1) * BLK)))
        ft = np.ascontiguousarray(
            np.stack([chunk_cols(ft2, cols) for cols in slot_cols], axis=0))
        sqc = np.ascontiguousarray(
            np.concatenate([sqd[cols] for cols in slot_cols])[None, :].astype(ml_dtypes.bfloat16))
        sqa = np.ascontiguousarray(
            -(sq[c * BLK:(c + 1) * BLK] - sbar).reshape(RT, 128).T.astype(np.float32))
        in_maps.append({"qt": qt, "ft": ft, "sqc": sqc, "sqa": sqa})

    nc = _build_nc()
    res = run_bass_kernel_spmd(nc, in_maps, core_ids=list(range(NCORES)), trace=TRACE)
    LAST_EXEC_NS = res.exec_time_ns

    # host merge: per global 128-row chunk, gather its candidate sets
    from collections import defaultdict
    chunk_sets = defaultdict(list)
    for c in range(NCORES):
        arr = np.asarray(res.results[c]["cand"]).reshape(128, NSETS, 8)
        slot_cols = col_tables[c]
        # computed sets: rows are always the core's own rows
        for i, g in enumerate(GROUPS):
            r = g[2] if g[0] == "full" else g[1]
            chunk_sets[(c * BLK) // 128 + r].append(arr[:, i, :])
        # mirror banks: rows = source columns of the transposed tiles
        for j, mb in enumerate(MBANKS):
            i = N_COMP + j
            if mb[0] in ("mfa", "mfb"):
                _, bi, q = mb
                col0 = slot_cols[bi][q * 128]
            elif mb[0] == "mk":
                _, h, q = mb
                col0 = slot_cols[3][h * 512 + q * 128]
            else:
                _, q = mb
                col0 = slot_cols[4][512 + q * 128]
            assert col0 % 128 == 0
            chunk_sets[col0 // 128].append(arr[:, i, :])

    t6 = np.empty(N, dtype=np.float32)
    for ch in range(N // 128):
        sets = chunk_sets[ch]
        assert len(sets) in (12, 13), (ch, len(sets))
        vals = np.concatenate(sets, axis=1)            # [128, 80]
        t6[ch * 128:(ch + 1) * 128] = np.partition(
            vals, vals.shape[1] - 1 - K_ORD, axis=1)[:, vals.shape[1] - 1 - K_ORD]
    kd = np.maximum((sq + sbar) - t6, 0.0)
    dens = 1.0 / (np.sqrt(kd) + EPS)
    return dens.astype(np.float32)[:, None]


# revision 3
# speedup vs baseline: 1.0952x; 1.0113x over previous
"""Symmetric brute-force KNN density estimator on 8 Trainium2 NeuronCores.

reference math:
    dist[i, j] = ||x_i - x_j||_2 over features [8192, 1024]
    kth[i] = 6th smallest of dist[i, :]  (self-distance included)
    out[i] = 1 / (kth[i] + 1e-8)

v3 strategy — full symmetry (circulant blocks + quadrant splits + host merge):
    Rank rows by T[i,j] = 2G[i,j] - (sq[j] - sbar); d2 = (sq[i]+sbar) - T.
    8 row-blocks of 1024. Core c computes, via fp8 DoubleRow matmuls:
      * blocks c+1..c+3 fully (24 [128,1024] PSUM groups), each mirrored;
      * the c+4 block's diagonal quadrants (8 [128,512] half-groups, all
        mirrored) — the paired core covers the anti-diagonal quadrants (its
        ft buffer has the halves swapped on the host);
      * the diagonal block's UL/LR quadrants fully plus UR mirrored
        (12 half-groups).
    Mirroring: the scalar engine copies raw-2G PSUM to SBUF bf16 adding the
    per-partition bias -(sq_i - sbar) (exactly the transposed tile's column
    bias), then dma_start_transpose scatters [128,128] pieces into mirror
    storage with a contiguous destination per source tile (a strided dest
    produces wrong output on HW). DVE max8 scans computed PSUM groups
    (FD=1024) and mirror banks (strided SBUF bf16 reads). The host merges
    80 top-8 candidate sets per core (10 sets per row, asserted) and does
    the sqrt/reciprocal recovery with exact fp32 norms.
"""

import os

import numpy as np
import ml_dtypes

N = 8192
D = 1024
NCORES = 8
NB = 8            # row/col blocks of 1024
BLK = N // NB     # 1024
KC = D // 128     # 8 contraction chunks -> 4 DoubleRow pairs
KP = KC // 2      # 4
RT = BLK // 128   # 8 row chunks per block
K_ORD = 5         # 6th largest/smallest
EPS = 1e-8
WARMUP_MM = 18
NSLOT = 5         # ft column-block slots: c+1, c+2, c+3, c+4, c(diag)
NMIR = 3          # full-block mirrored slots

# canonical device-order work lists (identical for every core; SPMD)
#   ('full', bi, r)          : [128,1024] group, slot bi in 0..2, mirrored
#   ('k4',   r)              : [128,512] group, slot 3, col off (r//4)*512, mirrored
#   ('dq',   r, off, mir)    : diag slot 4 half-group
GROUPS = (
    [("k4", r) for r in range(RT)]
    + [("dfull", r) for r in range(4)]
    + [("full", bi, r) for bi in range(NMIR) for r in range(RT)]
    + [("dq", r, 512, False) for r in range(4, 8)]
)
# mirror banks (scanned after their source transposes land). Full-block
# banks are split into r 0..3 / 4..7 halves so the first half drains while
# the block's later groups still run.
MBANKS = (
    [("mk", h, q) for h in range(2) for q in range(4)]
    + [("md", q) for q in range(4)]
    + [("mfa", bi, q) for bi in range(NMIR) for q in range(RT)]
    + [("mfb", bi, q) for bi in range(NMIR) for q in range(RT)]
)
N_COMP = len(GROUPS)            # 44
NSETS = N_COMP + len(MBANKS)    # 44 + 36 = 80

TRACE = bool(int(os.environ.get("KNN_TRACE", "0")))
LAST_EXEC_NS = None


def _build_nc():
    import concourse.mybir as mybir
    from concourse import bacc
    from concourse.tile import TileContext

    dt = mybir.dt
    nc = bacc.Bacc(None, target_bir_lowering=False, enable_partition_id=False)

    qt_d = nc.dram_tensor("qt", [128, KP * 2 * BLK], dt.float8e4, kind="ExternalInput")
    ft_d = nc.dram_tensor("ft", [NSLOT, 128, KP * 2 * BLK], dt.float8e4, kind="ExternalInput")
    sqc_d = nc.dram_tensor("sqc", [1, NSLOT * BLK], dt.bfloat16, kind="ExternalInput")
    sqa_d = nc.dram_tensor("sqa", [128, RT], dt.float32, kind="ExternalInput")
    cand_d = nc.dram_tensor("cand", [128, NSETS * 8], dt.float32, kind="ExternalOutput")

    DR = mybir.MatmulPerfMode.DoubleRow

    with TileContext(nc) as tc:
        with (
            tc.tile_pool(name="persist", bufs=1) as persist,
            tc.tile_pool(name="cp", bufs=3) as cpp,
            tc.tile_pool(name="cph", bufs=4) as cphp,
            tc.tile_pool(name="psum", bufs=4, space="PSUM") as psum,
        ):
            qt_s = persist.tile([128, KP, 2, BLK], dt.float8e4)
            ft_s = persist.tile([128, NSLOT, KP, 2, BLK], dt.float8e4)
            sqc_s = persist.tile([1, NSLOT * BLK], dt.bfloat16)
            sqa_s = persist.tile([128, RT], dt.float32)
            ones_s = persist.tile([1, 128], dt.bfloat16)
            warm_s = persist.tile([128, 512], dt.bfloat16)
            # contiguous-per-source-tile mirror storage (see module docstring)
            mir_f = persist.tile([128, NMIR, RT, RT, 128], dt.bfloat16)
            mir_k = persist.tile([128, RT, 4, 128], dt.bfloat16)
            mir_d = persist.tile([128, 4, 4, 128], dt.bfloat16)
            cand = persist.tile([128, NSETS * 8], dt.float32)

            # PE warm-up during the initial DMA window (HAM clock gate)
            nc.vector.memset(ones_s, 1.0)
            nc.vector.memset(warm_s, 0.0)
            wtile = psum.tile([128, 1024], dt.float32, tag="ps")
            for i in range(WARMUP_MM):
                nc.tensor.matmul(wtile[:, 0:512], lhsT=warm_s[:, 0:128], rhs=warm_s,
                                 start=(i == 0), stop=(i == WARMUP_MM - 1))

            qt_r = qt_d[:, :].rearrange("p (k t j) -> p k t j", k=KP, t=2)
            ft_r = [ft_d[b].rearrange("p (k t j) -> p k t j", k=KP, t=2)
                    for b in range(NSLOT)]
            nc.sync.dma_start(qt_s[:, 0:2], qt_r[:, 0:2])
            nc.sync.dma_start(ft_s[:, 3, 0:2], ft_r[3][:, 0:2])
            nc.sync.dma_start(qt_s[:, 2:4], qt_r[:, 2:4])
            nc.sync.dma_start(ft_s[:, 3, 2:4], ft_r[3][:, 2:4])
            nc.sync.dma_start(sqc_s, sqc_d[:, :])
            nc.sync.dma_start(sqa_s, sqa_d[:, :])
            nc.sync.dma_start(ft_s[:, 4], ft_r[4])
            nc.sync.dma_start(ft_s[:, 0], ft_r[0])
            nc.gpsimd.dma_start(ft_s[:, 1], ft_r[1])
            nc.gpsimd.dma_start(ft_s[:, 2], ft_r[2])

            # device emission must match the canonical order: computed sets
            # first (GROUPS order), then mirror banks (MBANKS order). Mirror
            # scans are interleaved for pipelining but their cand slots are
            # pre-assigned from the MBANKS order.
            comp_slots = {}
            for i, g in enumerate(GROUPS):
                comp_slots[g] = i
            bank_slots = {}
            for i, mb in enumerate(MBANKS):
                bank_slots[mb] = N_COMP + i

            def scan_bank_at(mb):
                s = bank_slots[mb]
                out = cand[:, s * 8:(s + 1) * 8]
                kind = mb[0]
                if kind == "mfa":
                    _, bi, q = mb
                    nc.vector.max(out=out, in_=mir_f[:, bi, 0:4, q, :])
                elif kind == "mfb":
                    _, bi, q = mb
                    nc.vector.max(out=out, in_=mir_f[:, bi, 4:8, q, :])
                elif kind == "mk":
                    _, h, q = mb
                    nc.vector.max(out=out, in_=mir_k[:, 4 * h:4 * h + 4, q, :])
                else:
                    _, q = mb
                    nc.vector.max(out=out, in_=mir_d[:, 0:4, q, :])

            tcount = [0]

            def do_group(g):
                kind = g[0]
                if kind == "full":
                    _, bi, r = g
                    slot, off, w, mir = bi, 0, 1024, True
                elif kind == "k4":
                    _, r = g
                    slot, off, w, mir = 3, (r // 4) * 512, 512, True
                elif kind == "dfull":
                    _, r = g
                    slot, off, w, mir = 4, 0, 1024, True
                else:
                    _, r, off, mir = g
                    slot, w = 4, 512
                ps = psum.tile([128, 1024], dt.float32, tag="ps")
                nh = w // 512
                for kp in range(KP):
                    for half in range(nh):
                        nc.tensor.matmul(
                            ps[:, half * 512:(half + 1) * 512],
                            lhsT=qt_s[:, kp, :, r * 128:(r + 1) * 128],
                            rhs=ft_s[:, slot, kp, :, off + half * 512: off + (half + 1) * 512],
                            start=(kp == 0), stop=False, perf_mode=DR)
                # fold the computed tile's column bias immediately (PE never
                # waits); the mirror copy below then carries an extra
                # -(sq_j-sbar) which is per-partition in the mirror bank, so
                # it does not affect ranking and the host adds it back.
                for half in range(nh):
                    c0 = slot * BLK + off + half * 512
                    nc.tensor.matmul(
                        ps[:, half * 512:(half + 1) * 512],
                        lhsT=ones_s, rhs=sqc_s[:, c0:c0 + 512],
                        start=False, stop=(half == nh - 1))
                if mir:
                    if kind == "full":
                        cp = cpp.tile([128, 1024], dt.bfloat16, tag="cp")
                        nc.scalar.add(cp, ps, sqa_s[:, r:r + 1])
                        dst = mir_f[:, slot, r, :, :]
                    elif kind == "dfull":
                        # only the UR half (cols 512:1024) is mirrored -> LL
                        cp = cphp.tile([128, 512], dt.bfloat16, tag="cph")
                        nc.scalar.add(cp, ps[:, 512:1024], sqa_s[:, r:r + 1])
                        dst = mir_d[:, r, :, :]
                    else:
                        cp = cphp.tile([128, 512], dt.bfloat16, tag="cph")
                        nc.scalar.add(cp, ps[:, 0:512], sqa_s[:, r:r + 1])
                        dst = mir_k[:, r, :, :] if kind == "k4" else mir_d[:, r, :, :]
                    if kind == "full":
                        eng = nc.sync if (tcount[0] % 2 == 0) else nc.scalar
                        tcount[0] += 1
                    else:
                        eng = nc.sync
                    eng.dma_start_transpose(dst, cp)
                s = comp_slots[g]
                nc.vector.max(out=cand[:, s * 8:(s + 1) * 8], in_=ps[:, 0:w])

            # schedule: emit groups; after each group, drain one pending bank
            ready_after = {}
            for bi in range(NMIR):
                ready_after[("full", bi, 3)] = [("mfa", bi, q) for q in range(RT)]
                ready_after[("full", bi, 7)] = [("mfb", bi, q) for q in range(RT)]
            ready_after[("k4", 3)] = [("mk", 0, q) for q in range(4)]
            ready_after[("k4", 7)] = [("mk", 1, q) for q in range(4)]
            ready_after[("dfull", 3)] = [("md", q) for q in range(4)]

            queue = []
            delay = []         # one-group delay before banks become poppable
            for g in GROUPS:
                do_group(g)
                queue.extend(delay)
                delay = ready_after.get(g, [])
                npop = 2 if len(queue) >= 4 else 1
                for _ in range(min(npop, len(queue))):
                    scan_bank_at(queue.pop(0))
            queue.extend(delay)
            for mb in queue:
                scan_bank_at(mb)

            nc.sync.dma_start(cand_d[:, :], cand)

    nc.finalize()
    return nc


def kernel(features):
    global LAST_EXEC_NS
    from concourse.bass_utils import run_bass_kernel_spmd

    f32 = np.ascontiguousarray(np.asarray(features, dtype=np.float32))
    assert f32.shape == (N, D)

    sq = np.einsum("nd,nd->n", f32, f32, dtype=np.float32)
    sbar = float(sq.mean())

    ftq = f32.T.astype(ml_dtypes.float8_e4m3fn)                   # [D, N] fp8
    ft2 = (ftq.astype(np.float32) * 2.0).astype(ml_dtypes.float8_e4m3fn)
    sqd = (-(sq - sbar)).astype(np.float32)

    def chunk_cols(src, cols):
        blk = src[:, cols]                                        # [D, BLK]
        return blk.reshape(KP, 2, 128, BLK).transpose(2, 0, 1, 3).reshape(128, KP * 2 * BLK)

    in_maps = []
    col_tables = []
    for c in range(NCORES):
        blocks = [(c + o) % NB for o in [1, 2, 3, 4, 0]]
        # slot 3 (k4): swap column halves for c >= 4 so the shared device
        # program computes complementary quadrants on the two paired cores
        slot_cols = []
        for si, b in enumerate(blocks):
            cols = np.arange(b * BLK, (b + 1) * BLK)
            if si == 3 and c >= 4:
                cols = np.concatenate([cols[512:], cols[:512]])
            slot_cols.append(cols)
        col_tables.append(slot_cols)
        qt = np.ascontiguousarray(chunk_cols(ftq, np.arange(c * BLK, (c + 1) * BLK)))
        ft = np.ascontiguousarray(
            np.stack([chunk_cols(ft2, cols) for cols in slot_cols], axis=0))
        sqc = np.ascontiguousarray(
            np.concatenate([sqd[cols] for cols in slot_cols])[None, :].astype(ml_dtypes.bfloat16))
        sqa = np.ascontiguousarray(
            -(sq[c * BLK:(c + 1) * BLK] - sbar).reshape(RT, 128).T.astype(np.float32))
        in_maps.append({"qt": qt, "ft": ft, "sqc": sqc, "sqa": sqa})

    nc = _build_nc()
    res = run_bass_kernel_spmd(nc, in_maps, core_ids=list(range(NCORES)), trace=TRACE)
    LAST_EXEC_NS = res.exec_time_ns

    # host merge: per global 128-row chunk, gather its candidate sets
    from collections import defaultdict
    chunk_sets = defaultdict(list)
    for c in range(NCORES):
        arr = np.asarray(res.results[c]["cand"]).reshape(128, NSETS, 8)
        slot_cols = col_tables[c]
        # computed sets: rows are always the core's own rows
        for i, g in enumerate(GROUPS):
            r = g[2] if g[0] == "full" else g[1]
            chunk_sets[(c * BLK) // 128 + r].append(arr[:, i, :])
        # mirror banks: rows = source columns of the transposed tiles
        for j, mb in enumerate(MBANKS):
            i = N_COMP + j
            if mb[0] in ("mfa", "mfb"):
                _, bi, q = mb
                col0 = slot_cols[bi][q * 128]
            elif mb[0] == "mk":
                _, h, q = mb
                col0 = slot_cols[3][h * 512 + q * 128]
            else:
                _, q = mb
                col0 = slot_cols[4][512 + q * 128]
            assert col0 % 128 == 0
            adj = (sq[col0:col0 + 128] - sbar).astype(np.float32)
            chunk_sets[col0 // 128].append(arr[:, i, :] + adj[:, None])

    t6 = np.empty(N, dtype=np.float32)
    for ch in range(N // 128):
        sets = chunk_sets[ch]
        assert len(sets) in (12, 13), (ch, len(sets))
        vals = np.concatenate(sets, axis=1)            # [128, 80]
        t6[ch * 128:(ch + 1) * 128] = np.partition(
            vals, vals.shape[1] - 1 - K_ORD, axis=1)[:, vals.shape[1] - 1 - K_ORD]
    kd = np.maximum((sq + sbar) - t6, 0.0)
    dens = 1.0 / (np.sqrt(kd) + EPS)
    return dens.astype(np.float32)[:, None]


# revision 4
# speedup vs baseline: 1.1214x; 1.0240x over previous
"""Symmetric brute-force KNN density estimator on 8 Trainium2 NeuronCores.

reference math:
    dist[i, j] = ||x_i - x_j||_2 over features [8192, 1024]
    kth[i] = 6th smallest of dist[i, :]  (self-distance included)
    out[i] = 1 / (kth[i] + 1e-8)

v3 strategy — full symmetry (circulant blocks + quadrant splits + host merge):
    Rank rows by T[i,j] = 2G[i,j] - (sq[j] - sbar); d2 = (sq[i]+sbar) - T.
    8 row-blocks of 1024. Core c computes, via fp8 DoubleRow matmuls:
      * blocks c+1..c+3 fully (24 [128,1024] PSUM groups), each mirrored;
      * the c+4 block's diagonal quadrants (8 [128,512] half-groups, all
        mirrored) — the paired core covers the anti-diagonal quadrants (its
        ft buffer has the halves swapped on the host);
      * the diagonal block's UL/LR quadrants fully plus UR mirrored
        (12 half-groups).
    Mirroring: the scalar engine copies raw-2G PSUM to SBUF bf16 adding the
    per-partition bias -(sq_i - sbar) (exactly the transposed tile's column
    bias), then dma_start_transpose scatters [128,128] pieces into mirror
    storage with a contiguous destination per source tile (a strided dest
    produces wrong output on HW). DVE max8 scans computed PSUM groups
    (FD=1024) and mirror banks (strided SBUF bf16 reads). The host merges
    80 top-8 candidate sets per core (10 sets per row, asserted) and does
    the sqrt/reciprocal recovery with exact fp32 norms.
"""

import os

import numpy as np
import ml_dtypes

N = 8192
D = 1024
NCORES = 8
NB = 8            # row/col blocks of 1024
BLK = N // NB     # 1024
KC = D // 128     # 8 contraction chunks -> 4 DoubleRow pairs
KP = KC // 2      # 4
RT = BLK // 128   # 8 row chunks per block
K_ORD = 5         # 6th largest/smallest
EPS = 1e-8
WARMUP_MM = 18
NSLOT = 5         # ft column-block slots: c+1, c+2, c+3, c+4, c(diag)
NMIR = 3          # full-block mirrored slots

# canonical device-order work lists (identical for every core; SPMD)
#   ('full', bi, r)          : [128,1024] group, slot bi in 0..2, mirrored
#   ('k4',   r)              : [128,512] group, slot 3, col off (r//4)*512, mirrored
#   ('dq',   r, off, mir)    : diag slot 4 half-group
GROUPS = (
    [("k4", r) for r in range(RT)]
    + [("dfull", r) for r in range(4)]
    + [("full", bi, r) for bi in range(NMIR) for r in range(RT)]
    + [("dq", r, 512, False) for r in range(4, 8)]
)
# mirror banks (scanned after their source transposes land). Full-block
# banks are split into r 0..3 / 4..7 halves so the first half drains while
# the block's later groups still run.
MBANKS = (
    [("mk", h, q) for h in range(2) for q in range(4)]
    + [("md", q) for q in range(4)]
    + [("mfa", bi, q) for bi in range(2) for q in range(RT)]
    + [("mfb", bi, q) for bi in range(2) for q in range(RT)]
    + [("mq", j2, q) for j2 in range(4) for q in range(RT)]
)
N_COMP = len(GROUPS)            # 44
NSETS = N_COMP + len(MBANKS)    # 44 + 36 = 80

TRACE = bool(int(os.environ.get("KNN_TRACE", "0")))
LAST_EXEC_NS = None


def _build_nc():
    import concourse.mybir as mybir
    from concourse import bacc
    from concourse.tile import TileContext

    dt = mybir.dt
    nc = bacc.Bacc(None, target_bir_lowering=False, enable_partition_id=False)

    qt_d = nc.dram_tensor("qt", [128, KP * 2 * BLK], dt.float8e4, kind="ExternalInput")
    ft_d = nc.dram_tensor("ft", [NSLOT, 128, KP * 2 * BLK], dt.float8e4, kind="ExternalInput")
    sqc_d = nc.dram_tensor("sqc", [1, NSLOT * BLK], dt.bfloat16, kind="ExternalInput")
    sqa_d = nc.dram_tensor("sqa", [128, RT], dt.float32, kind="ExternalInput")
    cand_d = nc.dram_tensor("cand", [128, NSETS * 8], dt.float32, kind="ExternalOutput")

    DR = mybir.MatmulPerfMode.DoubleRow

    with TileContext(nc) as tc:
        with (
            tc.tile_pool(name="persist", bufs=1) as persist,
            tc.tile_pool(name="cp", bufs=3) as cpp,
            tc.tile_pool(name="cph", bufs=4) as cphp,
            tc.tile_pool(name="psum", bufs=4, space="PSUM") as psum,
        ):
            qt_s = persist.tile([128, KP, 2, BLK], dt.float8e4)
            ft_s = persist.tile([128, NSLOT, KP, 2, BLK], dt.float8e4)
            sqc_s = persist.tile([1, NSLOT * BLK], dt.bfloat16)
            sqa_s = persist.tile([128, RT], dt.float32)
            ones_s = persist.tile([1, 128], dt.bfloat16)
            warm_s = persist.tile([128, 512], dt.bfloat16)
            # contiguous-per-source-tile mirror storage (see module docstring)
            mir_f = persist.tile([128, NMIR, RT, RT, 128], dt.bfloat16)
            mir_k = persist.tile([128, RT, 4, 128], dt.bfloat16)
            mir_d = persist.tile([128, 4, 4, 128], dt.bfloat16)
            cand = persist.tile([128, NSETS * 8], dt.float32)

            # PE warm-up during the initial DMA window (HAM clock gate)
            nc.vector.memset(ones_s, 1.0)
            nc.vector.memset(warm_s, 0.0)
            wtile = psum.tile([128, 1024], dt.float32, tag="ps")
            for i in range(WARMUP_MM):
                nc.tensor.matmul(wtile[:, 0:512], lhsT=warm_s[:, 0:128], rhs=warm_s,
                                 start=(i == 0), stop=(i == WARMUP_MM - 1))

            qt_r = qt_d[:, :].rearrange("p (k t j) -> p k t j", k=KP, t=2)
            ft_r = [ft_d[b].rearrange("p (k t j) -> p k t j", k=KP, t=2)
                    for b in range(NSLOT)]
            nc.sync.dma_start(qt_s[:, 0:2], qt_r[:, 0:2])
            nc.sync.dma_start(ft_s[:, 3, 0:2], ft_r[3][:, 0:2])
            nc.sync.dma_start(qt_s[:, 2:4], qt_r[:, 2:4])
            nc.sync.dma_start(ft_s[:, 3, 2:4], ft_r[3][:, 2:4])
            nc.sync.dma_start(sqc_s, sqc_d[:, :])
            nc.sync.dma_start(sqa_s, sqa_d[:, :])
            nc.sync.dma_start(ft_s[:, 4], ft_r[4])
            nc.sync.dma_start(ft_s[:, 0], ft_r[0])
            nc.gpsimd.dma_start(ft_s[:, 1], ft_r[1])
            nc.gpsimd.dma_start(ft_s[:, 2], ft_r[2])

            # device emission must match the canonical order: computed sets
            # first (GROUPS order), then mirror banks (MBANKS order). Mirror
            # scans are interleaved for pipelining but their cand slots are
            # pre-assigned from the MBANKS order.
            comp_slots = {}
            for i, g in enumerate(GROUPS):
                comp_slots[g] = i
            bank_slots = {}
            for i, mb in enumerate(MBANKS):
                bank_slots[mb] = N_COMP + i

            def scan_bank_at(mb):
                s = bank_slots[mb]
                out = cand[:, s * 8:(s + 1) * 8]
                kind = mb[0]
                if kind == "mfa":
                    _, bi, q = mb
                    nc.vector.max(out=out, in_=mir_f[:, bi, 0:4, q, :])
                elif kind == "mfb":
                    _, bi, q = mb
                    nc.vector.max(out=out, in_=mir_f[:, bi, 4:8, q, :])
                elif kind == "mq":
                    _, j2, q = mb
                    nc.vector.max(out=out, in_=mir_f[:, 2, 2 * j2:2 * j2 + 2, q, :])
                elif kind == "mk":
                    _, h, q = mb
                    nc.vector.max(out=out, in_=mir_k[:, 4 * h:4 * h + 4, q, :])
                else:
                    _, q = mb
                    nc.vector.max(out=out, in_=mir_d[:, 0:4, q, :])

            tcount = [0]

            def do_group(g):
                kind = g[0]
                if kind == "full":
                    _, bi, r = g
                    slot, off, w, mir = bi, 0, 1024, True
                elif kind == "k4":
                    _, r = g
                    slot, off, w, mir = 3, (r // 4) * 512, 512, True
                elif kind == "dfull":
                    _, r = g
                    slot, off, w, mir = 4, 0, 1024, True
                else:
                    _, r, off, mir = g
                    slot, w = 4, 512
                ps = psum.tile([128, 1024], dt.float32, tag="ps")
                nh = w // 512
                for kp in range(KP):
                    for half in range(nh):
                        nc.tensor.matmul(
                            ps[:, half * 512:(half + 1) * 512],
                            lhsT=qt_s[:, kp, :, r * 128:(r + 1) * 128],
                            rhs=ft_s[:, slot, kp, :, off + half * 512: off + (half + 1) * 512],
                            start=(kp == 0), stop=False, perf_mode=DR)
                # fold the computed tile's column bias immediately (PE never
                # waits); the mirror copy below then carries an extra
                # -(sq_j-sbar) which is per-partition in the mirror bank, so
                # it does not affect ranking and the host adds it back.
                for half in range(nh):
                    c0 = slot * BLK + off + half * 512
                    nc.tensor.matmul(
                        ps[:, half * 512:(half + 1) * 512],
                        lhsT=ones_s, rhs=sqc_s[:, c0:c0 + 512],
                        start=False, stop=(half == nh - 1))
                if mir:
                    if kind == "full":
                        cp = cpp.tile([128, 1024], dt.bfloat16, tag="cp")
                        nc.scalar.add(cp, ps, sqa_s[:, r:r + 1])
                        dst = mir_f[:, slot, r, :, :]
                    elif kind == "dfull":
                        # only the UR half (cols 512:1024) is mirrored -> LL
                        cp = cphp.tile([128, 512], dt.bfloat16, tag="cph")
                        nc.scalar.add(cp, ps[:, 512:1024], sqa_s[:, r:r + 1])
                        dst = mir_d[:, r, :, :]
                    else:
                        cp = cphp.tile([128, 512], dt.bfloat16, tag="cph")
                        nc.scalar.add(cp, ps[:, 0:512], sqa_s[:, r:r + 1])
                        dst = mir_k[:, r, :, :] if kind == "k4" else mir_d[:, r, :, :]
                    if kind == "full":
                        eng = nc.sync if (tcount[0] % 2 == 0) else nc.scalar
                        tcount[0] += 1
                    else:
                        eng = nc.sync
                    eng.dma_start_transpose(dst, cp)
                s = comp_slots[g]
                nc.vector.max(out=cand[:, s * 8:(s + 1) * 8], in_=ps[:, 0:w])

            # schedule: emit groups; after each group, drain one pending bank
            ready_after = {}
            for bi in range(2):
                ready_after[("full", bi, 3)] = [("mfa", bi, q) for q in range(RT)]
                ready_after[("full", bi, 7)] = [("mfb", bi, q) for q in range(RT)]
            for j2 in range(4):
                ready_after[("full", 2, 2 * j2 + 1)] = [("mq", j2, q) for q in range(RT)]
            ready_after[("k4", 3)] = [("mk", 0, q) for q in range(4)]
            ready_after[("k4", 7)] = [("mk", 1, q) for q in range(4)]
            ready_after[("dfull", 3)] = [("md", q) for q in range(4)]

            queue = []
            delay = []         # one-group delay before banks become poppable
            for g in GROUPS:
                do_group(g)
                queue.extend(delay)
                delay = ready_after.get(g, [])
                budget = 1250
                while queue and budget > 0:
                    budget -= 400 if queue[0][0] == "mq" else 660
                    scan_bank_at(queue.pop(0))
            queue.extend(delay)
            for mb in queue:
                scan_bank_at(mb)

            nc.sync.dma_start(cand_d[:, :], cand)

    nc.finalize()
    return nc


def kernel(features):
    global LAST_EXEC_NS
    from concourse.bass_utils import run_bass_kernel_spmd

    f32 = np.ascontiguousarray(np.asarray(features, dtype=np.float32))
    assert f32.shape == (N, D)

    sq = np.einsum("nd,nd->n", f32, f32, dtype=np.float32)
    sbar = float(sq.mean())

    ftq = f32.T.astype(ml_dtypes.float8_e4m3fn)                   # [D, N] fp8
    ft2 = (ftq.astype(np.float32) * 2.0).astype(ml_dtypes.float8_e4m3fn)
    sqd = (-(sq - sbar)).astype(np.float32)

    def chunk_cols(src, cols):
        blk = src[:, cols]                                        # [D, BLK]
        return blk.reshape(KP, 2, 128, BLK).transpose(2, 0, 1, 3).reshape(128, KP * 2 * BLK)

    in_maps = []
    col_tables = []
    for c in range(NCORES):
        blocks = [(c + o) % NB for o in [1, 2, 3, 4, 0]]
        # slot 3 (k4): swap column halves for c >= 4 so the shared device
        # program computes complementary quadrants on the two paired cores
        slot_cols = []
        for si, b in enumerate(blocks):
            cols = np.arange(b * BLK, (b + 1) * BLK)
            if si == 3 and c >= 4:
                cols = np.concatenate([cols[512:], cols[:512]])
            slot_cols.append(cols)
        col_tables.append(slot_cols)
        qt = np.ascontiguousarray(chunk_cols(ftq, np.arange(c * BLK, (c + 1) * BLK)))
        ft = np.ascontiguousarray(
            np.stack([chunk_cols(ft2, cols) for cols in slot_cols], axis=0))
        sqc = np.ascontiguousarray(
            np.concatenate([sqd[cols] for cols in slot_cols])[None, :].astype(ml_dtypes.bfloat16))
        sqa = np.ascontiguousarray(
            -(sq[c * BLK:(c + 1) * BLK] - sbar).reshape(RT, 128).T.astype(np.float32))
        in_maps.append({"qt": qt, "ft": ft, "sqc": sqc, "sqa": sqa})

    nc = _build_nc()
    res = run_bass_kernel_spmd(nc, in_maps, core_ids=list(range(NCORES)), trace=TRACE)
    LAST_EXEC_NS = res.exec_time_ns

    # host merge: per global 128-row chunk, gather its candidate sets
    from collections import defaultdict
    chunk_sets = defaultdict(list)
    for c in range(NCORES):
        arr = np.asarray(res.results[c]["cand"]).reshape(128, NSETS, 8)
        slot_cols = col_tables[c]
        # computed sets: rows are always the core's own rows
        for i, g in enumerate(GROUPS):
            r = g[2] if g[0] == "full" else g[1]
            chunk_sets[(c * BLK) // 128 + r].append(arr[:, i, :])
        # mirror banks: rows = source columns of the transposed tiles
        for j, mb in enumerate(MBANKS):
            i = N_COMP + j
            if mb[0] in ("mfa", "mfb"):
                _, bi, q = mb
                col0 = slot_cols[bi][q * 128]
            elif mb[0] == "mq":
                _, j2, q = mb
                col0 = slot_cols[2][q * 128]
            elif mb[0] == "mk":
                _, h, q = mb
                col0 = slot_cols[3][h * 512 + q * 128]
            else:
                _, q = mb
                col0 = slot_cols[4][512 + q * 128]
            assert col0 % 128 == 0
            adj = (sq[col0:col0 + 128] - sbar).astype(np.float32)
            chunk_sets[col0 // 128].append(arr[:, i, :] + adj[:, None])

    t6 = np.empty(N, dtype=np.float32)
    for ch in range(N // 128):
        sets = chunk_sets[ch]
        assert len(sets) in (14, 15), (ch, len(sets))
        vals = np.concatenate(sets, axis=1)            # [128, 80]
        t6[ch * 128:(ch + 1) * 128] = np.partition(
            vals, vals.shape[1] - 1 - K_ORD, axis=1)[:, vals.shape[1] - 1 - K_ORD]
    kd = np.maximum((sq + sbar) - t6, 0.0)
    dens = 1.0 / (np.sqrt(kd) + EPS)
    return dens.astype(np.float32)[:, None]
